# revision 16
# baseline (speedup 1.0000x reference)
"""Mie scattering phase function on 8 Trainium2 NeuronCores.

Math: the reference's S1/S2 amplitudes are polynomials in mu of degree <= NMAX+1.
We parity-split S(mu) = E(mu^2) + mu*O(mu^2) and fit the 8 scalar targets
(E/O parts of Re/Im S1/S2, pre-scaled by 1/sqrt(2 x^2)) in a two-level Chebyshev
product basis on uh = 2 mu^2 - 1:  { T_j(T_8(uh)) * T_r(uh) : j,r in 0..7 }
(64 features, spans degree <= 63).

Device kernel per core (131072 angles, 2 megatiles of 128x512 angle columns):
  - DVE: Chebyshev recurrences (fp32) + 49 feature products -> fp16 bands,
    stored band-major (unit-stride writes); ACT casts the 14 base bands.
  - PE transpose (identity matmul) of [128 angles, 2cols x 64 bands] -> PSUM,
    copyback to SBUF (alternating GPSIMD/ACT), then ONE matmul per column
    pair: lhsT = F^T [128=(2x64 bands), 128 angles], rhs = packed C [128, 16]
    (block-diagonal: rows 0-63 -> outputs 0-7 for even col, rows 64-127 ->
    outputs 8-15 for odd col). fp16 throughout; PSUM accumulates fp32.
  - DVE/ACT epilogue per 2-bank drain group: S_o = E_o + mu*O_o,
    phase = sum S_o^2 (scale folded into C).
No xbar DMA transposes (the old bottleneck: ~1.4us each on HW).
"""
import math
from contextlib import ExitStack

import numpy as np

NMAX = 135
R = 8
J = 8
NFEAT = R * J  # 64
N_ANGLES = 1048576
N_CORES = 8
PER_CORE = N_ANGLES // N_CORES  # 131072
P = 128
MEGA_COLS = 512                   # angle columns per megatile
N_MEGA = PER_CORE // (P * MEGA_COLS)  # 2
NUNIT = MEGA_COLS // 2            # column pairs per megatile
UNITS_PER_DRAIN = 64              # pairs per PSUM drain group (2 banks)
N_DRAIN = NUNIT // UNITS_PER_DRAIN  # 4


# ----------------------------------------------------------------------------
# Host-side math (float64): replicate the reference recurrences + basis fit
# ----------------------------------------------------------------------------

def _riccati_f64(z):
    z = complex(z)
    inv = 1.0 / (z + 1e-12)
    psi = np.zeros(NMAX + 2, np.complex128)
    chi = np.zeros(NMAX + 2, np.complex128)
    psi[0] = np.sin(z)
    psi[1] = psi[0] * inv - np.cos(z)
    chi[0] = -np.cos(z)
    chi[1] = np.cos(z) * inv - np.sin(z)
    for n in range(2, NMAX + 2):
        coef = (2.0 * n - 1.0) * inv
        psi[n] = coef * psi[n - 1] - psi[n - 2]
        chi[n] = coef * chi[n - 1] - chi[n - 2]
    xi = psi - 1j * chi
    i = np.arange(1, NMAX + 1, dtype=np.float64)
    psi_prime = np.concatenate([[np.cos(z)], psi[:NMAX] - i * inv * psi[1:NMAX + 1]])
    xi_prime = np.concatenate(
        [[np.cos(z) + 1j * np.sin(z)], xi[:NMAX] - i * inv * xi[1:NMAX + 1]])
    return psi, xi, psi_prime, xi_prime


def _mie_ab_f64(x, m):
    psi, xi, psip, xip = _riccati_f64(x)
    psm, _, psmp, _ = _riccati_f64(m * x)
    s = slice(1, NMAX + 1)
    an = (m * psm[s] * psip[s] - psi[s] * psmp[s]) / \
         (m * psm[s] * xip[s] - xi[s] * psmp[s] + 1e-30)
    bn = (psm[s] * psip[s] - m * psi[s] * psmp[s]) / \
         (psm[s] * xip[s] - m * xi[s] * psmp[s] + 1e-30)
    return an, bn


def _s1s2_f64(mu, x, m):
    an, bn = _mie_ab_f64(x, m)
    n = np.arange(1, NMAX + 1, dtype=np.float64)
    f = (2.0 * n + 1.0) / (n * (n + 1.0))
    fa, fb = f * an, f * bn
    mu = np.asarray(mu, np.float64)
    pi_all = np.zeros((NMAX, mu.size))
    tau_all = np.zeros((NMAX, mu.size))
    pi_all[0] = 3.0 * mu
    tau_all[0] = mu
    p1, p2 = 3.0 * mu, np.ones_like(mu)
    for k in range(2, NMAX + 1):
        nn = float(k)
        p = ((2 * nn + 1) * mu * p1 - (nn + 1) * p2) / nn
        t = nn * mu * p - (nn + 1) * p1
        pi_all[k - 1] = p
        tau_all[k - 1] = t
        p1, p2 = p, p1
    S1 = fa @ pi_all + fb @ tau_all
    S2 = fa @ tau_all + fb @ pi_all
    return S1, S2


def _cheb_T(k, x):
    return np.cos(k * np.arccos(np.clip(x, -1.0, 1.0)))


def _fit_coeffs(wavelength, radius, m_real, m_imag):
    """Returns C (NFEAT, 8) float64 — columns: E/O of S1r,S1i,S2r,S2i scaled."""
    x = 2.0 * math.pi * radius / (wavelength * 1e-9)
    m = m_real + 1j * m_imag
    M = 1024
    uh = np.cos((np.arange(M) + 0.5) * np.pi / M)
    u = (uh + 1.0) / 2.0
    mu = np.sqrt(u)
    S1p, S2p = _s1s2_f64(mu, x, m)
    S1m, S2m = _s1s2_f64(-mu, x, m)
    scale = 1.0 / math.sqrt(2.0 * x * x)
    targets = []
    pairs = ((S1p.real, S1m.real), (S1p.imag, S1m.imag),
             (S2p.real, S2m.real), (S2p.imag, S2m.imag))
    for Sp, Sm in pairs:                       # E parts: columns 0..3
        targets.append((Sp + Sm) / 2.0 * scale)
    for Sp, Sm in pairs:                       # O parts: columns 4..7
        targets.append((Sp - Sm) / (2.0 * mu + 1e-300) * scale)
    T = np.stack([_cheb_T(r, uh) for r in range(R + 1)])
    Tv = np.stack([_cheb_T(j, T[R]) for j in range(J)])
    B = np.zeros((M, NFEAT))
    for j in range(J):
        for r in range(R):
            B[:, j * R + r] = Tv[j] * T[r]
    C, *_ = np.linalg.lstsq(B, np.stack(targets, axis=1), rcond=None)
    return C


# ----------------------------------------------------------------------------
# Device kernel (Bass / Tile)
# ----------------------------------------------------------------------------

_CACHE = {}


def _build_nc(reps=1, skip=()):
    skip = set(skip)
    import concourse.bass as bass
    import concourse.mybir as mybir
    import concourse.tile as tile
    from concourse import bacc, masks

    f32 = mybir.dt.float32
    f16 = mybir.dt.float16
    AOP = mybir.AluOpType

    nc = bacc.Bacc("TRN2", target_bir_lowering=False, debug=False)
    mu_d = nc.dram_tensor("mu", [PER_CORE], f32, kind="ExternalInput").ap()
    cpk_d = nc.dram_tensor("cpk", [P, 16], f16, kind="ExternalInput").ap()
    ph_d = nc.dram_tensor("phase", [PER_CORE], f32, kind="ExternalOutput").ap()

    mu_v = mu_d.rearrange("(m p c) -> m p c", p=P, c=MEGA_COLS)
    ph_v = ph_d.rearrange("(m p c) -> m p c", p=P, c=MEGA_COLS)

    with tile.TileContext(nc) as tc, ExitStack() as ctx:
        const_p = ctx.enter_context(tc.tile_pool(name="const", bufs=1))
        fstore_p = ctx.enter_context(tc.tile_pool(name="fstore", bufs=1))
        tbase_p = ctx.enter_context(tc.tile_pool(name="tbase", bufs=1))
        mu_p = ctx.enter_context(tc.tile_pool(name="mu", bufs=2))
        ph_p = ctx.enter_context(tc.tile_pool(name="ph", bufs=2))
        tmp_p = ctx.enter_context(tc.tile_pool(name="tmp", bufs=3))
        sq_p = ctx.enter_context(tc.tile_pool(name="sq", bufs=4))
        ftT_p = ctx.enter_context(tc.tile_pool(name="ftT", bufs=4))
        psum_mm = ctx.enter_context(tc.tile_pool(name="psmm", bufs=2, space="PSUM"))
        psum_tr = ctx.enter_context(tc.tile_pool(name="pstr", bufs=3, space="PSUM"))

        cpk_sb = const_p.tile([P, 16], f16)
        nc.sync.dma_start(cpk_sb[:], cpk_d)
        ident = const_p.tile([P, P], f16)
        masks.make_identity(nc, ident[:])

        # guards: last epilogue instruction that read each psum_mm buf
        guard = [None, None]

        rep_cm = tc.For_i(0, reps, 1) if reps > 1 else None
        if rep_cm is not None:
            rep_cm.__enter__()
        for mt in range(N_MEGA):
            if mt > 0 or reps > 1:
                # collapse cross-megatile wait fan-in (HW wait-slot limit)
                tc.strict_bb_all_engine_barrier()
            mu_t = mu_p.tile([P, MEGA_COLS], f32)
            nc.sync.dma_start(mu_t[:], mu_v[mt])

            # --- base Chebyshev recurrences (fp32, DVE) ---
            # T_0..T_7(uh) stored r-inner: tb8[p, c*8 + r]; T_8 = v separate.
            tb8 = tbase_p.tile([P, 8 * MEGA_COLS], f32, tag="tb8")
            tb8v = tb8[:].rearrange("p (c r) -> p c r", r=8)
            t8 = tbase_p.tile([P, MEGA_COLS], f32, tag="t8")
            tvb = tbase_p.tile([P, 6 * MEGA_COLS], f32, tag="tvb")
            tvb3 = tvb[:].rearrange("p (j c) -> p j c", c=MEGA_COLS)

            u_t = tmp_p.tile([P, MEGA_COLS], f32, tag="utile")
            w_t = tmp_p.tile([P, MEGA_COLS], f32, tag="wtile")
            wv_t = tmp_p.tile([P, MEGA_COLS], f32, tag="wvtile")
            nc.vector.tensor_mul(u_t[:], mu_t[:], mu_t[:])
            nc.vector.memset(tb8v[:, :, 0], 1.0)  # T_0 = 1
            # T_1 = uh = 2u - 1 ; w = 2*uh = 4u - 2
            nc.vector.tensor_scalar(tb8v[:, :, 1], u_t[:], 2.0, -1.0,
                                    AOP.mult, AOP.add)
            nc.vector.tensor_scalar(w_t[:], u_t[:], 4.0, -2.0, AOP.mult, AOP.add)
            # T_r = w*T_{r-1} - T_{r-2}
            for r in range(2, R + 1):
                m_t = tmp_p.tile([P, MEGA_COLS], f32, tag="mrec")
                nc.vector.tensor_mul(m_t[:], w_t[:], tb8v[:, :, r - 1])
                dst = t8[:] if r == R else tb8v[:, :, r]
                nc.vector.tensor_sub(dst, m_t[:], tb8v[:, :, r - 2])
            # wv = 2*v ; Tv_j = wv*Tv_{j-1} - Tv_{j-2} (Tv_1 = v = t8)
            nc.vector.tensor_scalar(wv_t[:], t8[:], 2.0, None, AOP.mult)
            for j in range(2, J):
                pm1 = t8[:] if j == 2 else tvb3[:, j - 3]
                m_t = tmp_p.tile([P, MEGA_COLS], f32, tag="mrec")
                nc.vector.tensor_mul(m_t[:], wv_t[:], pm1)
                if j == 2:
                    nc.vector.tensor_scalar(tvb3[:, j - 2], m_t[:], -1.0,
                                            None, AOP.add)
                else:
                    pm2 = t8[:] if j == 3 else tvb3[:, j - 4]
                    nc.vector.tensor_sub(tvb3[:, j - 2], m_t[:], pm2)

            # --- features: fp16, k-inner store F[p, c*64 + j*8 + r] ---
            # one op per j: F5[:, j] = Tv_j (bcast over r) * T_{0..7}
            F = fstore_p.tile([P, NFEAT * MEGA_COLS], f16)
            F5 = F[:].rearrange("p (c j r) -> p j c r", j=J, r=8)
            if "feat" not in skip:
                nc.scalar.copy(F5[:, 0], tb8v[:])  # j=0: Tv_0 = 1
                for j in range(1, J):
                    tv_ap = t8[:] if j == 1 else tvb3[:, j - 2]
                    tv_b = tv_ap.rearrange("p (c one) -> p c one",
                                           one=1).broadcast_to(
                        [P, MEGA_COLS, 8])
                    eng = nc.gpsimd if j <= 5 else nc.vector
                    eng.tensor_mul(F5[:, j], tv_b, tb8v[:])

            # --- per column pair: PE transpose -> copyback -> matmul ---
            # transpose input: unit q = columns (2q, 2q+1), contiguous 128 bands
            F4 = F[:].rearrange("p (q m) -> p q m", m=2 * NFEAT)
            ps = None
            if "tr" in skip:
                ftT_static = ftT_p.tile([P, P], f16, tag="static")
                nc.vector.memset(ftT_static[:], 0.25)
            for q in range(NUNIT):
                g, u = q // UNITS_PER_DRAIN, q % UNITS_PER_DRAIN
                if u == 0:
                    ps = psum_mm.tile([P, UNITS_PER_DRAIN * 16], f32)
                    ps4 = ps[:].rearrange("p (u b o) -> p u b o", b=2, o=8)
                if "tr" not in skip:
                    pst = psum_tr.tile([P, P], f16)
                    nc.tensor.transpose(pst[:], F4[:, q], ident[:])
                    ftT = ftT_p.tile([P, P], f16)
                    if q % 2 == 0:
                        nc.vector.tensor_copy(ftT[:], pst[:])
                    else:
                        nc.scalar.copy(ftT[:], pst[:])
                else:
                    ftT = ftT_static
                if "mm" in skip:
                    continue
                start = (u % 32 == 0)
                mm = nc.tensor.matmul(ps4[:, u], ftT[:], cpk_sb[:],
                                      start=start, stop=(u % 32 == 31))
                buf = g % 2
                if start and guard[buf] is not None:
                    tile.add_dep_helper(mm.ins, guard[buf].ins, sync=True,
                                        reason="bank reused after epilogue")
                if not start:
                    tile.add_dep_helper(mm.ins, prev_mm.ins, sync=False,
                                        reason="psum bank order")
                prev_mm = mm

                if u == UNITS_PER_DRAIN - 1:
                    # --- drain epilogue for group g: cols 128g..128g+127 ---
                    cs = slice(P * g, P * (g + 1))
                    if "mm" in skip:
                        continue
                    mu3 = mu_t[:, cs].rearrange("p (u b) -> p u b", b=2)
                    s_t = sq_p.tile([P, P], f32, tag="stile")
                    q_t = sq_p.tile([P, P], f32, tag="qtile")
                    s3 = s_t[:].rearrange("p (u b) -> p u b", b=2)
                    q3 = q_t[:].rearrange("p (u b) -> p u b", b=2)
                    if g == 0:
                        ph_t = ph_p.tile([P, MEGA_COLS], f32)
                    ph3 = ph_t[:, cs].rearrange("p (u b) -> p u b", b=2)
                    for o in range(4):
                        E_o = ps4[:, :, :, o]
                        O_o = ps4[:, :, :, 4 + o]
                        nc.vector.tensor_mul(s3, mu3, O_o)
                        guard[g % 2] = nc.vector.tensor_add(s3, s3, E_o)
                        if o == 0:
                            nc.scalar.square(ph3, s3)
                        else:
                            nc.scalar.square(q3, s3)
                            nc.vector.tensor_add(ph3, ph3, q3)
                    if g == N_DRAIN - 1:
                        nc.sync.dma_start(ph_v[mt], ph_t[:])

        if rep_cm is not None:
            rep_cm.__exit__(None, None, None)

    nc.compile()
    return nc


def _get_compiled():
    if "nc" not in _CACHE:
        _CACHE["nc"] = _build_nc()
    return _CACHE["nc"]


def _make_in_maps(mu, wavelength, radius, m_real, m_imag):
    C = _fit_coeffs(wavelength, radius, m_real, m_imag)
    cpk = np.zeros((P, 16), np.float16)
    cpk[0:NFEAT, 0:8] = C.astype(np.float16)
    cpk[NFEAT:2 * NFEAT, 8:16] = C.astype(np.float16)
    shards = mu.reshape(N_CORES, PER_CORE)
    return [{"mu": shards[i], "cpk": cpk} for i in range(N_CORES)]


def kernel(cos_theta, wavelength, radius, m_real, m_imag):
    from concourse.bass_utils import run_bass_kernel_spmd

    mu = np.asarray(cos_theta, np.float32).reshape(-1)
    assert mu.size == N_ANGLES
    in_maps = _make_in_maps(mu, float(np.asarray(wavelength)),
                            float(np.asarray(radius)),
                            float(np.asarray(m_real)),
                            float(np.asarray(m_imag)))
    nc = _get_compiled()
    import os
    trace = bool(os.environ.get("MIE_TRACE"))
    res = run_bass_kernel_spmd(nc, in_maps, list(range(N_CORES)), trace=trace)
    _CACHE["last_res"] = res
    out = np.concatenate([np.asarray(res.results[i]["phase"], np.float32)
                          for i in range(N_CORES)])
    return out


# revision 26
# speedup vs baseline: 3.8448x; 3.8448x over previous
"""Mie scattering phase function on 8 Trainium2 NeuronCores.

Math: the reference's S1/S2 amplitudes are polynomials in mu of degree <= NMAX+1.
We parity-split S(mu) = E(mu^2) + mu*O(mu^2) and fit the 8 scalar targets
(E/O parts of Re/Im S1/S2, pre-scaled by 1/sqrt(2 x^2)) in a two-level Chebyshev
product basis on uh = 2 mu^2 - 1:  { T_j(T_8(uh)) * T_r(uh) : j,r in 0..7 }
(64 features, spans degree <= 63).

Device kernel per core (131072 angles, 2 megatiles of 128x512 angle columns):
  - DVE: Chebyshev recurrences (fp32) + 49 feature products -> fp16 bands,
    stored band-major (unit-stride writes); ACT casts the 14 base bands.
  - PE transpose (identity matmul) of [128 angles, 2cols x 64 bands] -> PSUM,
    copyback to SBUF (alternating GPSIMD/ACT), then ONE matmul per column
    pair: lhsT = F^T [128=(2x64 bands), 128 angles], rhs = packed C [128, 16]
    (block-diagonal: rows 0-63 -> outputs 0-7 for even col, rows 64-127 ->
    outputs 8-15 for odd col). fp16 throughout; PSUM accumulates fp32.
  - DVE/ACT epilogue per 2-bank drain group: S_o = E_o + mu*O_o,
    phase = sum S_o^2 (scale folded into C).
No xbar DMA transposes (the old bottleneck: ~1.4us each on HW).
"""
import math
from contextlib import ExitStack

import numpy as np

NMAX = 135
R = 8
J = 8
NFEAT = R * J  # 64
N_ANGLES = 1048576
N_CORES = 8
PER_CORE = N_ANGLES // N_CORES  # 131072
P = 128
MEGA_COLS = 512                   # angle columns per megatile
N_MEGA = PER_CORE // (P * MEGA_COLS)  # 2
NUNIT = MEGA_COLS // 2            # column pairs per megatile
UNITS_PER_DRAIN = 64              # pairs per PSUM drain group (2 banks)
N_DRAIN = NUNIT // UNITS_PER_DRAIN  # 4


# ----------------------------------------------------------------------------
# Host-side math (float64): replicate the reference recurrences + basis fit
# ----------------------------------------------------------------------------

def _riccati_f64(z):
    z = complex(z)
    inv = 1.0 / (z + 1e-12)
    psi = np.zeros(NMAX + 2, np.complex128)
    chi = np.zeros(NMAX + 2, np.complex128)
    psi[0] = np.sin(z)
    psi[1] = psi[0] * inv - np.cos(z)
    chi[0] = -np.cos(z)
    chi[1] = np.cos(z) * inv - np.sin(z)
    for n in range(2, NMAX + 2):
        coef = (2.0 * n - 1.0) * inv
        psi[n] = coef * psi[n - 1] - psi[n - 2]
        chi[n] = coef * chi[n - 1] - chi[n - 2]
    xi = psi - 1j * chi
    i = np.arange(1, NMAX + 1, dtype=np.float64)
    psi_prime = np.concatenate([[np.cos(z)], psi[:NMAX] - i * inv * psi[1:NMAX + 1]])
    xi_prime = np.concatenate(
        [[np.cos(z) + 1j * np.sin(z)], xi[:NMAX] - i * inv * xi[1:NMAX + 1]])
    return psi, xi, psi_prime, xi_prime


def _mie_ab_f64(x, m):
    psi, xi, psip, xip = _riccati_f64(x)
    psm, _, psmp, _ = _riccati_f64(m * x)
    s = slice(1, NMAX + 1)
    an = (m * psm[s] * psip[s] - psi[s] * psmp[s]) / \
         (m * psm[s] * xip[s] - xi[s] * psmp[s] + 1e-30)
    bn = (psm[s] * psip[s] - m * psi[s] * psmp[s]) / \
         (psm[s] * xip[s] - m * xi[s] * psmp[s] + 1e-30)
    return an, bn


def _s1s2_f64(mu, x, m):
    an, bn = _mie_ab_f64(x, m)
    n = np.arange(1, NMAX + 1, dtype=np.float64)
    f = (2.0 * n + 1.0) / (n * (n + 1.0))
    fa, fb = f * an, f * bn
    mu = np.asarray(mu, np.float64)
    pi_all = np.zeros((NMAX, mu.size))
    tau_all = np.zeros((NMAX, mu.size))
    pi_all[0] = 3.0 * mu
    tau_all[0] = mu
    p1, p2 = 3.0 * mu, np.ones_like(mu)
    for k in range(2, NMAX + 1):
        nn = float(k)
        p = ((2 * nn + 1) * mu * p1 - (nn + 1) * p2) / nn
        t = nn * mu * p - (nn + 1) * p1
        pi_all[k - 1] = p
        tau_all[k - 1] = t
        p1, p2 = p, p1
    S1 = fa @ pi_all + fb @ tau_all
    S2 = fa @ tau_all + fb @ pi_all
    return S1, S2


def _cheb_T(k, x):
    return np.cos(k * np.arccos(np.clip(x, -1.0, 1.0)))


def _fit_coeffs(wavelength, radius, m_real, m_imag):
    """Returns C (NFEAT, 8) float64 — columns: E/O of S1r,S1i,S2r,S2i scaled."""
    x = 2.0 * math.pi * radius / (wavelength * 1e-9)
    m = m_real + 1j * m_imag
    M = 1024
    uh = np.cos((np.arange(M) + 0.5) * np.pi / M)
    u = (uh + 1.0) / 2.0
    mu = np.sqrt(u)
    S1p, S2p = _s1s2_f64(mu, x, m)
    S1m, S2m = _s1s2_f64(-mu, x, m)
    scale = 1.0 / math.sqrt(2.0 * x * x)
    targets = []
    pairs = ((S1p.real, S1m.real), (S1p.imag, S1m.imag),
             (S2p.real, S2m.real), (S2p.imag, S2m.imag))
    for Sp, Sm in pairs:                       # E parts: columns 0..3
        targets.append((Sp + Sm) / 2.0 * scale)
    for Sp, Sm in pairs:                       # O parts: columns 4..7
        targets.append((Sp - Sm) / (2.0 * mu + 1e-300) * scale)
    T = np.stack([_cheb_T(r, uh) for r in range(R + 1)])
    Tv = np.stack([_cheb_T(j, T[R]) for j in range(J)])
    B = np.zeros((M, NFEAT))
    for j in range(J):
        for r in range(R):
            B[:, j * R + r] = Tv[j] * T[r]
    C, *_ = np.linalg.lstsq(B, np.stack(targets, axis=1), rcond=None)
    return C


# ----------------------------------------------------------------------------
# Device kernel (Bass / Tile)
# ----------------------------------------------------------------------------

_CACHE = {}


def _build_nc(reps=1, skip=()):
    skip = set(skip)
    import concourse.bass as bass
    import concourse.mybir as mybir
    import concourse.tile as tile
    from concourse import bacc, masks

    f32 = mybir.dt.float32
    f16 = mybir.dt.float16
    AOP = mybir.AluOpType

    nc = bacc.Bacc("TRN2", target_bir_lowering=False, debug=False)
    mu_d = nc.dram_tensor("mu", [PER_CORE], f32, kind="ExternalInput").ap()
    cpk_d = nc.dram_tensor("cpk", [P, 16], f16, kind="ExternalInput").ap()
    ph_d = nc.dram_tensor("phase", [PER_CORE], f32, kind="ExternalOutput").ap()

    mu_v = mu_d.rearrange("(m p c) -> m p c", p=P, c=MEGA_COLS)
    ph_v = ph_d.rearrange("(m p c) -> m p c", p=P, c=MEGA_COLS)

    with tile.TileContext(nc) as tc, ExitStack() as ctx:
        const_p = ctx.enter_context(tc.tile_pool(name="const", bufs=1))
        fstore_p = ctx.enter_context(tc.tile_pool(name="fstore", bufs=2))
        tbase_p = ctx.enter_context(tc.tile_pool(name="tbase", bufs=1))
        mu_p = ctx.enter_context(tc.tile_pool(name="mu", bufs=2))
        ph_p = ctx.enter_context(tc.tile_pool(name="ph", bufs=2))
        tmp_p = ctx.enter_context(tc.tile_pool(name="tmp", bufs=2))
        sq_p = ctx.enter_context(tc.tile_pool(name="sq", bufs=2))
        ftT_p = ctx.enter_context(tc.tile_pool(name="ftT", bufs=3))
        psum_mm = ctx.enter_context(tc.tile_pool(name="psmm", bufs=2, space="PSUM"))
        psum_tr = ctx.enter_context(tc.tile_pool(name="pstr", bufs=2, space="PSUM"))

        cpk_sb = const_p.tile([P, 16], f16)
        nc.sync.dma_start(cpk_sb[:], cpk_d)
        ident = const_p.tile([P, P], f16)
        masks.make_identity(nc, ident[:])

        # guards: last epilogue instruction that read each psum_mm buf
        guard = [None, None]

        rep_cm = tc.For_i(0, reps, 1) if reps > 1 else None
        if rep_cm is not None:
            rep_cm.__enter__()
            if "bar" not in skip:
                # one barrier per rep: collapses cross-rep wait fan-in
                # (F bufs=2 covers both megatiles within a rep)
                tc.strict_bb_all_engine_barrier()
        for mt in range(N_MEGA):
            mu_t = mu_p.tile([P, MEGA_COLS], f32)
            nc.sync.dma_start(mu_t[:], mu_v[mt])

            # --- base Chebyshev recurrences (fp32, DVE) ---
            # T_0..T_7(uh) stored r-inner: tb8[p, c*8 + r]; T_8 = v separate.
            tb8 = tbase_p.tile([P, 8 * MEGA_COLS], f32, tag="tb8")
            tb8v = tb8[:].rearrange("p (c r) -> p c r", r=8)
            t8 = tbase_p.tile([P, MEGA_COLS], f32, tag="t8")
            tvb = tbase_p.tile([P, 6 * MEGA_COLS], f32, tag="tvb")
            tvb3 = tvb[:].rearrange("p (j c) -> p j c", c=MEGA_COLS)

            u_t = tmp_p.tile([P, MEGA_COLS], f32, tag="utile")
            w_t = tmp_p.tile([P, MEGA_COLS], f32, tag="wtile")
            wv_t = tmp_p.tile([P, MEGA_COLS], f32, tag="wvtile")
            if "rec" in skip:
                continue
            nc.vector.tensor_mul(u_t[:], mu_t[:], mu_t[:])
            nc.vector.memset(tb8v[:, :, 0], 1.0)  # T_0 = 1
            # T_1 = uh = 2u - 1 ; w = 2*uh = 4u - 2
            nc.vector.tensor_scalar(tb8v[:, :, 1], u_t[:], 2.0, -1.0,
                                    AOP.mult, AOP.add)
            nc.vector.tensor_scalar(w_t[:], u_t[:], 4.0, -2.0, AOP.mult, AOP.add)
            # T_r = w*T_{r-1} - T_{r-2}
            for r in range(2, R + 1):
                m_t = tmp_p.tile([P, MEGA_COLS], f32, tag="mrec")
                nc.vector.tensor_mul(m_t[:], w_t[:], tb8v[:, :, r - 1])
                dst = t8[:] if r == R else tb8v[:, :, r]
                nc.vector.tensor_sub(dst, m_t[:], tb8v[:, :, r - 2])
            # wv = 2*v ; Tv_j = wv*Tv_{j-1} - Tv_{j-2} (Tv_1 = v = t8)
            nc.vector.tensor_scalar(wv_t[:], t8[:], 2.0, None, AOP.mult)
            for j in range(2, J):
                pm1 = t8[:] if j == 2 else tvb3[:, j - 3]
                m_t = tmp_p.tile([P, MEGA_COLS], f32, tag="mrec")
                nc.vector.tensor_mul(m_t[:], wv_t[:], pm1)
                if j == 2:
                    nc.vector.tensor_scalar(tvb3[:, j - 2], m_t[:], -1.0,
                                            None, AOP.add)
                else:
                    pm2 = t8[:] if j == 3 else tvb3[:, j - 4]
                    nc.vector.tensor_sub(tvb3[:, j - 2], m_t[:], pm2)

            # --- features: fp16, k-inner store F[p, c*64 + j*8 + r] ---
            # one op per j: F5[:, j] = Tv_j (bcast over r) * T_{0..7}
            F = fstore_p.tile([P, NFEAT * MEGA_COLS], f16)
            F5 = F[:].rearrange("p (c j r) -> p j c r", j=J, r=8)
            if "feat" not in skip:
                nc.scalar.copy(F5[:, 0], tb8v[:])  # j=0: Tv_0 = 1
                for j in range(1, J):
                    tv_ap = t8[:] if j == 1 else tvb3[:, j - 2]
                    tv_b = tv_ap.rearrange("p (c one) -> p c one",
                                           one=1).broadcast_to(
                        [P, MEGA_COLS, 8])
                    eng = nc.gpsimd if j <= 5 else nc.vector
                    eng.tensor_mul(F5[:, j], tv_b, tb8v[:])

            # --- per column pair: PE transpose -> copyback -> matmul ---
            # transpose input: unit q = columns (2q, 2q+1), contiguous 128 bands
            F4 = F[:].rearrange("p (q m) -> p q m", m=2 * NFEAT)
            ps = None
            if "tr" in skip:
                ftT_static = ftT_p.tile([P, 4 * P], f16, tag="static")
                nc.vector.memset(ftT_static[:], 0.25)
            ftT4 = None
            for q in range(NUNIT):
                g, u = q // UNITS_PER_DRAIN, q % UNITS_PER_DRAIN
                s = q % 4
                if u == 0:
                    ps = psum_mm.tile([P, UNITS_PER_DRAIN * 16], f32)
                    ps4 = ps[:].rearrange("p (u b o) -> p u b o", b=2, o=8)
                if "tr" not in skip:
                    # 4 transposes share one PSUM bank; single batched copyback
                    if s == 0:
                        pst4 = psum_tr.tile([P, 4 * P], f16)
                        pst4v = pst4[:].rearrange("p (s a) -> p s a", s=4)
                    tr = nc.tensor.matmul(pst4v[:, s], F4[:, q], ident[:],
                                          is_transpose=True,
                                          start=(s == 0), stop=(s == 3))
                    if s > 0:
                        tile.add_dep_helper(tr.ins, prev_tr.ins, sync=False,
                                            reason="transpose bank order")
                    prev_tr = tr
                    if s == 3:
                        ftT4 = ftT_p.tile([P, 4 * P], f16)
                        if (q // 4) % 2 == 0:
                            nc.vector.tensor_copy(ftT4[:], pst4[:])
                        else:
                            nc.scalar.copy(ftT4[:], pst4[:])
                elif s == 3:
                    ftT4 = ftT_static
                if "mm" in skip:
                    continue
                if s == 3:
                    # issue the 4 matmuls for units q-3..q
                    for si in range(4):
                        qq = q - 3 + si
                        uu = qq % UNITS_PER_DRAIN
                        start = (uu % 32 == 0)
                        mm = nc.tensor.matmul(
                            ps4[:, uu], ftT4[:, si * P:(si + 1) * P], cpk_sb[:],
                            start=start, stop=(uu % 32 == 31))
                        if start and guard[g % 2] is not None:
                            tile.add_dep_helper(mm.ins, guard[g % 2].ins,
                                                sync=True,
                                                reason="bank reuse after epi")
                        if not start:
                            tile.add_dep_helper(mm.ins, prev_mm.ins, sync=False,
                                                reason="psum bank order")
                        prev_mm = mm

                if u == UNITS_PER_DRAIN - 1:
                    # --- drain epilogue for group g: cols 128g..128g+127 ---
                    # S_t = E_t + mu*O_t (batched over t); phase = 4*avg(S_t^2)
                    # (the 2x S-scale is folded into C host-side)
                    cs = slice(P * g, P * (g + 1))
                    mu4 = mu_t[:, cs].rearrange(
                        "p (u b one) -> p u b one", b=2, one=1).broadcast_to(
                        [P, UNITS_PER_DRAIN, 2, 4])
                    s_t = sq_p.tile([P, UNITS_PER_DRAIN * 8], f32, tag="stile")
                    s4 = s_t[:].rearrange("p (u b t) -> p u b t", b=2, t=4)
                    sq_t = sq_p.tile([P, UNITS_PER_DRAIN * 8], f32, tag="sqtile")
                    if g == 0:
                        ph_t = ph_p.tile([P, MEGA_COLS], f32)
                    nc.vector.tensor_mul(s4, mu4, ps4[:, :, :, 4:8])
                    guard[g % 2] = nc.vector.tensor_add(s4, s4, ps4[:, :, :, 0:4])
                    nc.scalar.square(sq_t[:], s_t[:])
                    sq4 = sq_t[:].rearrange("p (c t) -> p c t", t=4)
                    pr_t = sq_p.tile([P, UNITS_PER_DRAIN * 4], f32, tag="prtile")
                    pr3 = pr_t[:].rearrange("p (c t) -> p c t", t=2)
                    nc.vector.tensor_add(pr3, sq4[:, :, 0:2], sq4[:, :, 2:4])
                    nc.vector.tensor_add(ph_t[:, cs], pr3[:, :, 0], pr3[:, :, 1])
                    if g == N_DRAIN - 1:
                        nc.sync.dma_start(ph_v[mt], ph_t[:])

        if rep_cm is not None:
            rep_cm.__exit__(None, None, None)

    nc.compile()
    return nc


def _get_compiled():
    if "nc" not in _CACHE:
        _CACHE["nc"] = _build_nc()
    return _CACHE["nc"]


def _make_in_maps(mu, wavelength, radius, m_real, m_imag):
    C = _fit_coeffs(wavelength, radius, m_real, m_imag)
    cpk = np.zeros((P, 16), np.float16)
    cpk[0:NFEAT, 0:8] = C.astype(np.float16)
    cpk[NFEAT:2 * NFEAT, 8:16] = C.astype(np.float16)
    shards = mu.reshape(N_CORES, PER_CORE)
    return [{"mu": shards[i], "cpk": cpk} for i in range(N_CORES)]


def kernel(cos_theta, wavelength, radius, m_real, m_imag):
    from concourse.bass_utils import run_bass_kernel_spmd

    mu = np.asarray(cos_theta, np.float32).reshape(-1)
    assert mu.size == N_ANGLES
    in_maps = _make_in_maps(mu, float(np.asarray(wavelength)),
                            float(np.asarray(radius)),
                            float(np.asarray(m_real)),
                            float(np.asarray(m_imag)))
    nc = _get_compiled()
    import os
    trace = bool(os.environ.get("MIE_TRACE"))
    res = run_bass_kernel_spmd(nc, in_maps, list(range(N_CORES)), trace=trace)
    _CACHE["last_res"] = res
    out = np.concatenate([np.asarray(res.results[i]["phase"], np.float32)
                          for i in range(N_CORES)])
    return out


# revision 34
# speedup vs baseline: 3.8856x; 1.0106x over previous
"""Mie scattering phase function on 8 Trainium2 NeuronCores.

Math: the reference's S1/S2 amplitudes are polynomials in mu of degree <= NMAX+1.
We parity-split S(mu) = E(mu^2) + mu*O(mu^2) and fit the 8 scalar targets
(E/O parts of Re/Im S1/S2, pre-scaled by 1/sqrt(2 x^2)) in a two-level Chebyshev
product basis on uh = 2 mu^2 - 1:  { T_j(T_8(uh)) * T_r(uh) : j,r in 0..7 }
(64 features, spans degree <= 63).

Device kernel per core (131072 angles, 2 megatiles of 128x512 angle columns):
  - DVE: Chebyshev recurrences (fp32) + 49 feature products -> fp16 bands,
    stored band-major (unit-stride writes); ACT casts the 14 base bands.
  - PE transpose (identity matmul) of [128 angles, 2cols x 64 bands] -> PSUM,
    copyback to SBUF (alternating GPSIMD/ACT), then ONE matmul per column
    pair: lhsT = F^T [128=(2x64 bands), 128 angles], rhs = packed C [128, 16]
    (block-diagonal: rows 0-63 -> outputs 0-7 for even col, rows 64-127 ->
    outputs 8-15 for odd col). fp16 throughout; PSUM accumulates fp32.
  - DVE/ACT epilogue per 2-bank drain group: S_o = E_o + mu*O_o,
    phase = sum S_o^2 (scale folded into C).
No xbar DMA transposes (the old bottleneck: ~1.4us each on HW).
"""
import math
from contextlib import ExitStack

import numpy as np

NMAX = 135
R = 8
J = 8
NFEAT = R * J  # 64
N_ANGLES = 1048576
N_CORES = 8
PER_CORE = N_ANGLES // N_CORES  # 131072
P = 128
MEGA_COLS = 512                   # angle columns per megatile
N_MEGA = PER_CORE // (P * MEGA_COLS)  # 2
NUNIT = MEGA_COLS // 2            # column pairs per megatile
UNITS_PER_DRAIN = 64              # pairs per PSUM drain group (2 banks)
N_DRAIN = NUNIT // UNITS_PER_DRAIN  # 4


# ----------------------------------------------------------------------------
# Host-side math (float64): replicate the reference recurrences + basis fit
# ----------------------------------------------------------------------------

def _riccati_f64(z):
    z = complex(z)
    inv = 1.0 / (z + 1e-12)
    psi = np.zeros(NMAX + 2, np.complex128)
    chi = np.zeros(NMAX + 2, np.complex128)
    psi[0] = np.sin(z)
    psi[1] = psi[0] * inv - np.cos(z)
    chi[0] = -np.cos(z)
    chi[1] = np.cos(z) * inv - np.sin(z)
    for n in range(2, NMAX + 2):
        coef = (2.0 * n - 1.0) * inv
        psi[n] = coef * psi[n - 1] - psi[n - 2]
        chi[n] = coef * chi[n - 1] - chi[n - 2]
    xi = psi - 1j * chi
    i = np.arange(1, NMAX + 1, dtype=np.float64)
    psi_prime = np.concatenate([[np.cos(z)], psi[:NMAX] - i * inv * psi[1:NMAX + 1]])
    xi_prime = np.concatenate(
        [[np.cos(z) + 1j * np.sin(z)], xi[:NMAX] - i * inv * xi[1:NMAX + 1]])
    return psi, xi, psi_prime, xi_prime


def _mie_ab_f64(x, m):
    psi, xi, psip, xip = _riccati_f64(x)
    psm, _, psmp, _ = _riccati_f64(m * x)
    s = slice(1, NMAX + 1)
    an = (m * psm[s] * psip[s] - psi[s] * psmp[s]) / \
         (m * psm[s] * xip[s] - xi[s] * psmp[s] + 1e-30)
    bn = (psm[s] * psip[s] - m * psi[s] * psmp[s]) / \
         (psm[s] * xip[s] - m * xi[s] * psmp[s] + 1e-30)
    return an, bn


def _s1s2_f64(mu, x, m):
    an, bn = _mie_ab_f64(x, m)
    n = np.arange(1, NMAX + 1, dtype=np.float64)
    f = (2.0 * n + 1.0) / (n * (n + 1.0))
    fa, fb = f * an, f * bn
    mu = np.asarray(mu, np.float64)
    pi_all = np.zeros((NMAX, mu.size))
    tau_all = np.zeros((NMAX, mu.size))
    pi_all[0] = 3.0 * mu
    tau_all[0] = mu
    p1, p2 = 3.0 * mu, np.ones_like(mu)
    for k in range(2, NMAX + 1):
        nn = float(k)
        p = ((2 * nn + 1) * mu * p1 - (nn + 1) * p2) / nn
        t = nn * mu * p - (nn + 1) * p1
        pi_all[k - 1] = p
        tau_all[k - 1] = t
        p1, p2 = p, p1
    S1 = fa @ pi_all + fb @ tau_all
    S2 = fa @ tau_all + fb @ pi_all
    return S1, S2


def _cheb_T(k, x):
    return np.cos(k * np.arccos(np.clip(x, -1.0, 1.0)))


def _fit_coeffs(wavelength, radius, m_real, m_imag):
    """Returns C (NFEAT, 8) float64 — columns: E/O of S1r,S1i,S2r,S2i scaled."""
    x = 2.0 * math.pi * radius / (wavelength * 1e-9)
    m = m_real + 1j * m_imag
    M = 1024
    uh = np.cos((np.arange(M) + 0.5) * np.pi / M)
    u = (uh + 1.0) / 2.0
    mu = np.sqrt(u)
    S1p, S2p = _s1s2_f64(mu, x, m)
    S1m, S2m = _s1s2_f64(-mu, x, m)
    scale = 1.0 / math.sqrt(2.0 * x * x)
    targets = []
    pairs = ((S1p.real, S1m.real), (S1p.imag, S1m.imag),
             (S2p.real, S2m.real), (S2p.imag, S2m.imag))
    for Sp, Sm in pairs:                       # E parts: columns 0..3
        targets.append((Sp + Sm) / 2.0 * scale)
    for Sp, Sm in pairs:                       # O parts: columns 4..7
        targets.append((Sp - Sm) / (2.0 * mu + 1e-300) * scale)
    T = np.stack([_cheb_T(r, uh) for r in range(R + 1)])
    Tv = np.stack([_cheb_T(j, T[R]) for j in range(J)])
    B = np.zeros((M, NFEAT))
    for j in range(J):
        for r in range(R):
            B[:, j * R + r] = Tv[j] * T[r]
    C, *_ = np.linalg.lstsq(B, np.stack(targets, axis=1), rcond=None)
    return C


# ----------------------------------------------------------------------------
# Device kernel (Bass / Tile)
# ----------------------------------------------------------------------------

_CACHE = {}


def _build_nc(reps=1, skip=()):
    skip = set(skip)
    import concourse.bass as bass
    import concourse.mybir as mybir
    import concourse.tile as tile
    from concourse import bacc, masks

    f32 = mybir.dt.float32
    f16 = mybir.dt.float16
    AOP = mybir.AluOpType

    nc = bacc.Bacc("TRN2", target_bir_lowering=False, debug=False)
    mu_d = nc.dram_tensor("mu", [PER_CORE], f32, kind="ExternalInput").ap()
    cpk_d = nc.dram_tensor("cpk", [P, 16], f16, kind="ExternalInput").ap()
    ph_d = nc.dram_tensor("phase", [PER_CORE], f32, kind="ExternalOutput").ap()

    mu_v = mu_d.rearrange("(m p c) -> m p c", p=P, c=MEGA_COLS)
    ph_v = ph_d.rearrange("(m p c) -> m p c", p=P, c=MEGA_COLS)

    with tile.TileContext(nc) as tc, ExitStack() as ctx:
        const_p = ctx.enter_context(tc.tile_pool(name="const", bufs=1))
        fstore_p = ctx.enter_context(tc.tile_pool(name="fstore", bufs=2))
        tbase_p = ctx.enter_context(tc.tile_pool(name="tbase", bufs=1))
        mu_p = ctx.enter_context(tc.tile_pool(name="mu", bufs=2))
        ph_p = ctx.enter_context(tc.tile_pool(name="ph", bufs=2))
        tmp_p = ctx.enter_context(tc.tile_pool(name="tmp", bufs=2))
        sq_p = ctx.enter_context(tc.tile_pool(name="sq", bufs=2))
        ftT_p = ctx.enter_context(tc.tile_pool(name="ftT", bufs=3))
        psum_mm = ctx.enter_context(tc.tile_pool(name="psmm", bufs=2, space="PSUM"))
        psum_tr = ctx.enter_context(tc.tile_pool(name="pstr", bufs=2, space="PSUM"))

        cpk_sb = const_p.tile([P, 16], f16)
        nc.sync.dma_start(cpk_sb[:], cpk_d)
        ident = const_p.tile([P, P], f16)
        masks.make_identity(nc, ident[:])
        hpi_t = const_p.tile([P, 1], f32)
        nc.vector.memset(hpi_t[:], math.pi / 2.0)

        # guards: last epilogue instruction that read each psum_mm buf
        guard = [None, None]

        rep_cm = tc.For_i(0, reps, 1) if reps > 1 else None
        if rep_cm is not None:
            rep_cm.__enter__()
            if "bar" not in skip:
                # one barrier per rep: collapses cross-rep wait fan-in
                # (F bufs=2 covers both megatiles within a rep)
                tc.strict_bb_all_engine_barrier()
        for mt in range(N_MEGA):
            mu_t = mu_p.tile([P, MEGA_COLS], f32)
            nc.sync.dma_start(mu_t[:], mu_v[mt])

            # --- base Chebyshev recurrences (fp32, DVE) ---
            # T_0..T_7(uh) stored r-inner: tb8[p, c*8 + r]; T_8 = v separate.
            tb8 = tbase_p.tile([P, 8 * MEGA_COLS], f32, tag="tb8")
            tb8v = tb8[:].rearrange("p (c r) -> p c r", r=8)
            t8 = tbase_p.tile([P, MEGA_COLS], f32, tag="t8")
            tvb = tbase_p.tile([P, 6 * MEGA_COLS], f32, tag="tvb")
            tvb3 = tvb[:].rearrange("p (j c) -> p j c", c=MEGA_COLS)

            u_t = tmp_p.tile([P, MEGA_COLS], f32, tag="utile")
            if "rec" in skip:
                continue
            # Binary-doubling Chebyshev: T_2m = 2 T_m^2 - 1 (square on ACT,
            # affine on DVE); T_{m+n} = 2 T_m T_n - T_{m-n} (mul on GPSIMD,
            # fused affine-sub on DVE). Critical path ~14 ops across 3 engines
            # (vs 31 serial DVE ops for the linear recurrence).
            nc.vector.memset(tb8v[:, :, 0], 1.0)  # T_0 = 1
            nc.vector.tensor_mul(u_t[:], mu_t[:], mu_t[:])
            T = {}

            def TD(m):
                return T[m]

            def dbl(m, dst):
                # T_2m = 2 T_m^2 - 1
                sq = tmp_p.tile([P, MEGA_COLS], f32, tag=f"sq{(2*m) % 3}")
                nc.scalar.square(sq[:], TD(m))
                nc.vector.tensor_scalar(dst, sq[:], 2.0, -1.0, AOP.mult, AOP.add)
                T[2 * m] = dst

            def add_(m, n, dst):
                # T_{m+n} = 2 T_m T_n - T_{m-n}
                pr = tmp_p.tile([P, MEGA_COLS], f32, tag=f"pr{(m+n) % 3}")
                nc.gpsimd.tensor_mul(pr[:], TD(m), TD(n))
                nc.vector.scalar_tensor_tensor(dst, pr[:], 2.0, TD(m - n),
                                               AOP.mult, AOP.subtract)
                T[m + n] = dst

            # T_1 = 2u - 1
            nc.vector.tensor_scalar(tb8v[:, :, 1], u_t[:], 2.0, -1.0,
                                    AOP.mult, AOP.add)
            T[1] = tb8v[:, :, 1]
            dbl(1, tb8v[:, :, 2])
            add_(2, 1, tb8v[:, :, 3])
            dbl(2, tb8v[:, :, 4])
            add_(3, 2, tb8v[:, :, 5])
            dbl(3, tb8v[:, :, 6])
            add_(4, 3, tb8v[:, :, 7])
            dbl(4, t8[:])
            dbl(8, tvb3[:, 0])          # Tv_2 = T_16
            add_(16, 8, tvb3[:, 1])     # Tv_3 = T_24
            dbl(16, tvb3[:, 2])         # Tv_4 = T_32
            add_(24, 16, tvb3[:, 3])    # Tv_5 = T_40
            dbl(24, tvb3[:, 4])         # Tv_6 = T_48
            add_(32, 24, tvb3[:, 5])    # Tv_7 = T_56

            # --- features: fp16, k-inner store F[p, c*64 + j*8 + r] ---
            # one op per j: F5[:, j] = Tv_j (bcast over r) * T_{0..7}
            F = fstore_p.tile([P, NFEAT * MEGA_COLS], f16)
            F5 = F[:].rearrange("p (c j r) -> p j c r", j=J, r=8)
            if "feat" not in skip:
                nc.scalar.copy(F5[:, 0], tb8v[:])  # j=0: Tv_0 = 1
                for j in range(1, J):
                    tv_ap = t8[:] if j == 1 else tvb3[:, j - 2]
                    tv_b = tv_ap.rearrange("p (c one) -> p c one",
                                           one=1).broadcast_to(
                        [P, MEGA_COLS, 8])
                    eng = nc.gpsimd if j <= 5 else nc.vector
                    eng.tensor_mul(F5[:, j], tv_b, tb8v[:])

            # --- per column pair: PE transpose -> copyback -> matmul ---
            # transpose input: unit q = columns (2q, 2q+1), contiguous 128 bands
            F4 = F[:].rearrange("p (q m) -> p q m", m=2 * NFEAT)
            ps = None
            if "tr" in skip:
                ftT_static = ftT_p.tile([P, 4 * P], f16, tag="static")
                nc.vector.memset(ftT_static[:], 0.25)
            ftT4 = None
            for q in range(NUNIT):
                g, u = q // UNITS_PER_DRAIN, q % UNITS_PER_DRAIN
                s = q % 4
                if u == 0:
                    ps = psum_mm.tile([P, UNITS_PER_DRAIN * 16], f32)
                    ps4 = ps[:].rearrange("p (u b o) -> p u b o", b=2, o=8)
                if "tr" not in skip:
                    # 4 transposes share one PSUM bank; single batched copyback
                    if s == 0:
                        pst4 = psum_tr.tile([P, 4 * P], f16)
                        pst4v = pst4[:].rearrange("p (s a) -> p s a", s=4)
                    tr = nc.tensor.matmul(pst4v[:, s], F4[:, q], ident[:],
                                          is_transpose=True,
                                          start=(s == 0), stop=(s == 3))
                    if s > 0:
                        tile.add_dep_helper(tr.ins, prev_tr.ins, sync=False,
                                            reason="transpose bank order")
                    prev_tr = tr
                    if s == 3:
                        ftT4 = ftT_p.tile([P, 4 * P], f16)
                        if (q // 4) % 2 == 0:
                            nc.vector.tensor_copy(ftT4[:], pst4[:])
                        else:
                            nc.scalar.copy(ftT4[:], pst4[:])
                elif s == 3:
                    ftT4 = ftT_static
                if "mm" in skip:
                    continue
                if s == 3:
                    # issue the 4 matmuls for units q-3..q
                    for si in range(4):
                        qq = q - 3 + si
                        uu = qq % UNITS_PER_DRAIN
                        start = (uu % 32 == 0)
                        mm = nc.tensor.matmul(
                            ps4[:, uu], ftT4[:, si * P:(si + 1) * P], cpk_sb[:],
                            start=start, stop=(uu % 32 == 31))
                        if start and guard[g % 2] is not None:
                            tile.add_dep_helper(mm.ins, guard[g % 2].ins,
                                                sync=True,
                                                reason="bank reuse after epi")
                        if not start:
                            tile.add_dep_helper(mm.ins, prev_mm.ins, sync=False,
                                                reason="psum bank order")
                        prev_mm = mm

                if u == UNITS_PER_DRAIN - 1:
                    # --- drain epilogue for group g: cols 128g..128g+127 ---
                    # S_t = E_t + mu*O_t (batched over t); phase = 4*avg(S_t^2)
                    # (the 2x S-scale is folded into C host-side)
                    cs = slice(P * g, P * (g + 1))
                    mu4 = mu_t[:, cs].rearrange(
                        "p (u b one) -> p u b one", b=2, one=1).broadcast_to(
                        [P, UNITS_PER_DRAIN, 2, 4])
                    s_t = sq_p.tile([P, UNITS_PER_DRAIN * 8], f32, tag="stile")
                    s4 = s_t[:].rearrange("p (u b t) -> p u b t", b=2, t=4)
                    sq_t = sq_p.tile([P, UNITS_PER_DRAIN * 8], f32, tag="sqtile")
                    if g == 0:
                        ph_t = ph_p.tile([P, MEGA_COLS], f32)
                    nc.vector.tensor_mul(s4, mu4, ps4[:, :, :, 4:8])
                    guard[g % 2] = nc.vector.tensor_add(s4, s4, ps4[:, :, :, 0:4])
                    nc.scalar.square(sq_t[:], s_t[:])
                    sq4 = sq_t[:].rearrange("p (c t) -> p c t", t=4)
                    pr_t = sq_p.tile([P, UNITS_PER_DRAIN * 4], f32, tag="prtile")
                    pr3 = pr_t[:].rearrange("p (c t) -> p c t", t=2)
                    nc.vector.tensor_add(pr3, sq4[:, :, 0:2], sq4[:, :, 2:4])
                    nc.vector.tensor_add(ph_t[:, cs], pr3[:, :, 0], pr3[:, :, 1])
                    if g == N_DRAIN - 1:
                        nc.sync.dma_start(ph_v[mt], ph_t[:])

        if rep_cm is not None:
            rep_cm.__exit__(None, None, None)

    nc.compile()
    return nc


def _get_compiled():
    if "nc" not in _CACHE:
        _CACHE["nc"] = _build_nc()
    return _CACHE["nc"]


def _make_in_maps(mu, wavelength, radius, m_real, m_imag):
    C = _fit_coeffs(wavelength, radius, m_real, m_imag)
    cpk = np.zeros((P, 16), np.float16)
    cpk[0:NFEAT, 0:8] = C.astype(np.float16)
    cpk[NFEAT:2 * NFEAT, 8:16] = C.astype(np.float16)
    shards = mu.reshape(N_CORES, PER_CORE)
    return [{"mu": shards[i], "cpk": cpk} for i in range(N_CORES)]


def kernel(cos_theta, wavelength, radius, m_real, m_imag):
    from concourse.bass_utils import run_bass_kernel_spmd

    mu = np.asarray(cos_theta, np.float32).reshape(-1)
    assert mu.size == N_ANGLES
    in_maps = _make_in_maps(mu, float(np.asarray(wavelength)),
                            float(np.asarray(radius)),
                            float(np.asarray(m_real)),
                            float(np.asarray(m_imag)))
    nc = _get_compiled()
    import os
    trace = bool(os.environ.get("MIE_TRACE"))
    res = run_bass_kernel_spmd(nc, in_maps, list(range(N_CORES)), trace=trace)
    _CACHE["last_res"] = res
    out = np.concatenate([np.asarray(res.results[i]["phase"], np.float32)
                          for i in range(N_CORES)])
    return out


# revision 40
# speedup vs baseline: 6.7095x; 1.7268x over previous
"""Mie scattering phase function on 8 Trainium2 NeuronCores.

Math: the reference's S1/S2 amplitudes are polynomials in mu of degree <= NMAX+1.
We parity-split S(mu) = E(mu^2) + mu*O(mu^2) and fit the 8 scalar targets
(E/O parts of Re/Im S1/S2, pre-scaled by 1/sqrt(2 x^2)) in a two-level Chebyshev
product basis on uh = 2 mu^2 - 1:  { T_j(T_8(uh)) * T_r(uh) : j,r in 0..7 }
(64 features, spans degree <= 63).

Device kernel per core (131072 angles, 2 megatiles of 128x512 angle columns):
  - DVE: Chebyshev recurrences (fp32) + 49 feature products -> fp16 bands,
    stored band-major (unit-stride writes); ACT casts the 14 base bands.
  - PE transpose (identity matmul) of [128 angles, 2cols x 64 bands] -> PSUM,
    copyback to SBUF (alternating GPSIMD/ACT), then ONE matmul per column
    pair: lhsT = F^T [128=(2x64 bands), 128 angles], rhs = packed C [128, 16]
    (block-diagonal: rows 0-63 -> outputs 0-7 for even col, rows 64-127 ->
    outputs 8-15 for odd col). fp16 throughout; PSUM accumulates fp32.
  - DVE/ACT epilogue per 2-bank drain group: S_o = E_o + mu*O_o,
    phase = sum S_o^2 (scale folded into C).
No xbar DMA transposes (the old bottleneck: ~1.4us each on HW).
"""
import math
from contextlib import ExitStack

import numpy as np

NMAX = 135
R = 8
J = 8
NFEAT = R * J  # 64
N_ANGLES = 1048576
N_CORES = 8
PER_CORE = N_ANGLES // N_CORES  # 131072
P = 128
MEGA_COLS = 512                   # angle columns per megatile
N_MEGA = PER_CORE // (P * MEGA_COLS)  # 2
NUNIT = MEGA_COLS // 2            # column pairs per megatile
UNITS_PER_DRAIN = 64              # pairs per PSUM drain group (2 banks)
N_DRAIN = NUNIT // UNITS_PER_DRAIN  # 4


# ----------------------------------------------------------------------------
# Host-side math (float64): replicate the reference recurrences + basis fit
# ----------------------------------------------------------------------------

def _riccati_f64(z):
    z = complex(z)
    inv = 1.0 / (z + 1e-12)
    psi = np.zeros(NMAX + 2, np.complex128)
    chi = np.zeros(NMAX + 2, np.complex128)
    psi[0] = np.sin(z)
    psi[1] = psi[0] * inv - np.cos(z)
    chi[0] = -np.cos(z)
    chi[1] = np.cos(z) * inv - np.sin(z)
    for n in range(2, NMAX + 2):
        coef = (2.0 * n - 1.0) * inv
        psi[n] = coef * psi[n - 1] - psi[n - 2]
        chi[n] = coef * chi[n - 1] - chi[n - 2]
    xi = psi - 1j * chi
    i = np.arange(1, NMAX + 1, dtype=np.float64)
    psi_prime = np.concatenate([[np.cos(z)], psi[:NMAX] - i * inv * psi[1:NMAX + 1]])
    xi_prime = np.concatenate(
        [[np.cos(z) + 1j * np.sin(z)], xi[:NMAX] - i * inv * xi[1:NMAX + 1]])
    return psi, xi, psi_prime, xi_prime


def _mie_ab_f64(x, m):
    psi, xi, psip, xip = _riccati_f64(x)
    psm, _, psmp, _ = _riccati_f64(m * x)
    s = slice(1, NMAX + 1)
    an = (m * psm[s] * psip[s] - psi[s] * psmp[s]) / \
         (m * psm[s] * xip[s] - xi[s] * psmp[s] + 1e-30)
    bn = (psm[s] * psip[s] - m * psi[s] * psmp[s]) / \
         (psm[s] * xip[s] - m * xi[s] * psmp[s] + 1e-30)
    return an, bn


def _s1s2_f64(mu, x, m):
    an, bn = _mie_ab_f64(x, m)
    n = np.arange(1, NMAX + 1, dtype=np.float64)
    f = (2.0 * n + 1.0) / (n * (n + 1.0))
    fa, fb = f * an, f * bn
    mu = np.asarray(mu, np.float64)
    pi_all = np.zeros((NMAX, mu.size))
    tau_all = np.zeros((NMAX, mu.size))
    pi_all[0] = 3.0 * mu
    tau_all[0] = mu
    p1, p2 = 3.0 * mu, np.ones_like(mu)
    for k in range(2, NMAX + 1):
        nn = float(k)
        p = ((2 * nn + 1) * mu * p1 - (nn + 1) * p2) / nn
        t = nn * mu * p - (nn + 1) * p1
        pi_all[k - 1] = p
        tau_all[k - 1] = t
        p1, p2 = p, p1
    S1 = fa @ pi_all + fb @ tau_all
    S2 = fa @ tau_all + fb @ pi_all
    return S1, S2


def _cheb_T(k, x):
    return np.cos(k * np.arccos(np.clip(x, -1.0, 1.0)))


def _fit_coeffs(wavelength, radius, m_real, m_imag):
    """Returns C (NFEAT, 8) float64 — columns: E/O of S1r,S1i,S2r,S2i scaled."""
    x = 2.0 * math.pi * radius / (wavelength * 1e-9)
    m = m_real + 1j * m_imag
    M = 1024
    uh = np.cos((np.arange(M) + 0.5) * np.pi / M)
    u = (uh + 1.0) / 2.0
    mu = np.sqrt(u)
    S1p, S2p = _s1s2_f64(mu, x, m)
    S1m, S2m = _s1s2_f64(-mu, x, m)
    scale = 1.0 / math.sqrt(2.0 * x * x)
    targets = []
    pairs = ((S1p.real, S1m.real), (S1p.imag, S1m.imag),
             (S2p.real, S2m.real), (S2p.imag, S2m.imag))
    for Sp, Sm in pairs:                       # E parts: columns 0..3
        targets.append((Sp + Sm) / 2.0 * scale)
    for Sp, Sm in pairs:                       # O parts: columns 4..7
        targets.append((Sp - Sm) / (2.0 * mu + 1e-300) * scale)
    T = np.stack([_cheb_T(r, uh) for r in range(R + 1)])
    Tv = np.stack([_cheb_T(j, T[R]) for j in range(J)])
    B = np.zeros((M, NFEAT))
    for j in range(J):
        for r in range(R):
            B[:, j * R + r] = Tv[j] * T[r]
    C, *_ = np.linalg.lstsq(B, np.stack(targets, axis=1), rcond=None)
    return C


# ----------------------------------------------------------------------------
# Device kernel (Bass / Tile)
# ----------------------------------------------------------------------------

_CACHE = {}


def _build_nc(reps=1, skip=()):
    skip = set(skip)
    import concourse.bass as bass
    import concourse.mybir as mybir
    import concourse.tile as tile
    from concourse import bacc, masks

    f32 = mybir.dt.float32
    f16 = mybir.dt.float16
    AOP = mybir.AluOpType

    nc = bacc.Bacc("TRN2", target_bir_lowering=False, debug=False)
    mu_d = nc.dram_tensor("mu", [PER_CORE], f32, kind="ExternalInput").ap()
    cpk_d = nc.dram_tensor("cpk", [P, 16], f16, kind="ExternalInput").ap()
    ph_d = nc.dram_tensor("phase", [PER_CORE], f32, kind="ExternalOutput").ap()

    mu_v = mu_d.rearrange("(m p c) -> m p c", p=P, c=MEGA_COLS)
    ph_v = ph_d.rearrange("(m p c) -> m p c", p=P, c=MEGA_COLS)

    with tile.TileContext(nc) as tc, ExitStack() as ctx:
        const_p = ctx.enter_context(tc.tile_pool(name="const", bufs=1))
        fstore_p = ctx.enter_context(tc.tile_pool(name="fstore", bufs=2))
        tbase_p = ctx.enter_context(tc.tile_pool(name="tbase", bufs=1))
        mu_p = ctx.enter_context(tc.tile_pool(name="mu", bufs=2))
        ph_p = ctx.enter_context(tc.tile_pool(name="ph", bufs=2))
        tmp_p = ctx.enter_context(tc.tile_pool(name="tmp", bufs=1))
        sq_p = ctx.enter_context(tc.tile_pool(name="sq", bufs=2))
        ftT_p = ctx.enter_context(tc.tile_pool(name="ftT", bufs=2))
        psum_mm = ctx.enter_context(tc.tile_pool(name="psmm", bufs=2, space="PSUM"))
        psum_tr = ctx.enter_context(tc.tile_pool(name="pstr", bufs=2, space="PSUM"))

        cpk_sb = const_p.tile([P, 16], f16)
        nc.sync.dma_start(cpk_sb[:], cpk_d)
        ident = const_p.tile([P, P], f16)
        masks.make_identity(nc, ident[:])
        hpi_t = const_p.tile([P, 1], f32)
        nc.vector.memset(hpi_t[:], math.pi / 2.0)

        # guards: last epilogue instruction that read each psum_mm buf
        guard = [None, None]

        rep_cm = tc.For_i(0, reps, 1) if reps > 1 else None
        if rep_cm is not None:
            rep_cm.__enter__()
            if "bar" not in skip:
                # one barrier per rep: collapses cross-rep wait fan-in
                # (F bufs=2 covers both megatiles within a rep)
                tc.strict_bb_all_engine_barrier()
        for mt in range(N_MEGA):
            mu_t = mu_p.tile([P, MEGA_COLS], f32)
            nc.sync.dma_start(mu_t[:], mu_v[mt])

            # --- base Chebyshev recurrences (fp32, DVE) ---
            # T_0..T_7(uh) stored r-inner: tb8[p, c*8 + r]; T_8 = v separate.
            tb8 = tbase_p.tile([P, 8 * MEGA_COLS], f32, tag="tb8")
            tb8v = tb8[:].rearrange("p (c r) -> p c r", r=8)
            t8 = tbase_p.tile([P, MEGA_COLS], f32, tag="t8")
            tvb = tbase_p.tile([P, 6 * MEGA_COLS], f32, tag="tvb")
            tvb3 = tvb[:].rearrange("p (j c) -> p j c", c=MEGA_COLS)

            u_t = tmp_p.tile([P, MEGA_COLS], f32, tag="utile")
            if "rec" in skip:
                continue
            # Binary-doubling Chebyshev: T_2m = 2 T_m^2 - 1 (square on ACT,
            # affine on DVE); T_{m+n} = 2 T_m T_n - T_{m-n} (mul on GPSIMD,
            # fused affine-sub on DVE). Critical path ~14 ops across 3 engines
            # (vs 31 serial DVE ops for the linear recurrence).
            nc.vector.memset(tb8v[:, :, 0], 1.0)  # T_0 = 1
            nc.vector.tensor_mul(u_t[:], mu_t[:], mu_t[:])
            T = {}

            def TD(m):
                return T[m]

            def dbl(m, dst):
                # T_2m = 2 T_m^2 - 1
                sq = tmp_p.tile([P, MEGA_COLS], f32, tag=f"sq{(2*m) % 3}")
                nc.scalar.square(sq[:], TD(m))
                nc.vector.tensor_scalar(dst, sq[:], 2.0, -1.0, AOP.mult, AOP.add)
                T[2 * m] = dst

            def add_(m, n, dst):
                # T_{m+n} = 2 T_m T_n - T_{m-n}
                pr = tmp_p.tile([P, MEGA_COLS], f32, tag=f"pr{(m+n) % 3}")
                nc.gpsimd.tensor_mul(pr[:], TD(m), TD(n))
                nc.vector.scalar_tensor_tensor(dst, pr[:], 2.0, TD(m - n),
                                               AOP.mult, AOP.subtract)
                T[m + n] = dst

            # T_1 = 2u - 1
            nc.vector.tensor_scalar(tb8v[:, :, 1], u_t[:], 2.0, -1.0,
                                    AOP.mult, AOP.add)
            T[1] = tb8v[:, :, 1]
            dbl(1, tb8v[:, :, 2])
            add_(2, 1, tb8v[:, :, 3])
            dbl(2, tb8v[:, :, 4])
            add_(3, 2, tb8v[:, :, 5])
            dbl(3, tb8v[:, :, 6])
            add_(4, 3, tb8v[:, :, 7])
            dbl(4, t8[:])
            dbl(8, tvb3[:, 0])          # Tv_2 = T_16
            add_(16, 8, tvb3[:, 1])     # Tv_3 = T_24
            dbl(16, tvb3[:, 2])         # Tv_4 = T_32
            add_(24, 16, tvb3[:, 3])    # Tv_5 = T_40
            dbl(24, tvb3[:, 4])         # Tv_6 = T_48
            add_(32, 24, tvb3[:, 5])    # Tv_7 = T_56

            # --- features: fp16, k-inner store F[p, c*64 + j*8 + r] ---
            # one op per j: F5[:, j] = Tv_j (bcast over r) * T_{0..7}
            F = fstore_p.tile([P, NFEAT * MEGA_COLS], f16)
            F5 = F[:].rearrange("p (c j r) -> p j c r", j=J, r=8)
            if "feat" not in skip:
                nc.scalar.copy(F5[:, 0], tb8v[:])  # j=0: Tv_0 = 1
                for j in range(1, J):
                    tv_ap = t8[:] if j == 1 else tvb3[:, j - 2]
                    tv_b = tv_ap.rearrange("p (c one) -> p c one",
                                           one=1).broadcast_to(
                        [P, MEGA_COLS, 8])
                    eng = nc.gpsimd if j <= 4 else nc.vector
                    eng.tensor_mul(F5[:, j], tv_b, tb8v[:])

            # --- per column pair: PE transpose -> copyback -> matmul ---
            # transpose input: unit q = columns (2q, 2q+1), contiguous 128 bands
            F4 = F[:].rearrange("p (q m) -> p q m", m=2 * NFEAT)
            ps = None
            if "tr" in skip:
                ftT_static = ftT_p.tile([P, 8 * P], f16, tag="static")
                nc.vector.memset(ftT_static[:], 0.25)
            ftT4 = None
            for q in range(NUNIT):
                g, u = q // UNITS_PER_DRAIN, q % UNITS_PER_DRAIN
                s = q % 8
                if u == 0:
                    ps = psum_mm.tile([P, UNITS_PER_DRAIN * 16], f32)
                    ps4 = ps[:].rearrange("p (u b o) -> p u b o", b=2, o=8)
                if "tr" not in skip:
                    # 8 transposes share 2 PSUM banks; single batched copyback
                    if s == 0:
                        pst8 = psum_tr.tile([P, 8 * P], f16)
                        pst8v = pst8[:].rearrange("p (s a) -> p s a", s=8)
                    tr = nc.tensor.matmul(pst8v[:, s], F4[:, q], ident[:],
                                          is_transpose=True,
                                          start=(s % 4 == 0), stop=(s % 4 == 3))
                    if s % 4 != 0:
                        tile.add_dep_helper(tr.ins, prev_tr.ins, sync=False,
                                            reason="transpose bank order")
                    prev_tr = tr
                    if s == 7:
                        ftT8 = ftT_p.tile([P, 8 * P], f16)
                        if (q // 8) % 2 == 0:
                            nc.vector.tensor_copy(ftT8[:], pst8[:])
                        else:
                            nc.scalar.copy(ftT8[:], pst8[:])
                elif s == 7:
                    ftT8 = ftT_static
                if "mm" in skip:
                    continue
                if s == 7:
                    # issue the 8 matmuls for units q-7..q
                    for si in range(8):
                        qq = q - 7 + si
                        uu = qq % UNITS_PER_DRAIN
                        start = (uu % 32 == 0)
                        mm = nc.tensor.matmul(
                            ps4[:, uu], ftT8[:, si * P:(si + 1) * P], cpk_sb[:],
                            start=start, stop=(uu % 32 == 31))
                        if start and guard[g % 2] is not None:
                            tile.add_dep_helper(mm.ins, guard[g % 2].ins,
                                                sync=True,
                                                reason="bank reuse after epi")
                        if not start:
                            tile.add_dep_helper(mm.ins, prev_mm.ins, sync=False,
                                                reason="psum bank order")
                        prev_mm = mm

                if u == UNITS_PER_DRAIN - 1:
                    # --- drain epilogue for group g: cols 128g..128g+127 ---
                    # S_t = E_t + mu*O_t (batched over t); phase = 4*avg(S_t^2)
                    # (the 2x S-scale is folded into C host-side)
                    cs = slice(P * g, P * (g + 1))
                    mu4 = mu_t[:, cs].rearrange(
                        "p (u b one) -> p u b one", b=2, one=1).broadcast_to(
                        [P, UNITS_PER_DRAIN, 2, 4])
                    s_t = sq_p.tile([P, UNITS_PER_DRAIN * 8], f32, tag="stile")
                    s4 = s_t[:].rearrange("p (u b t) -> p u b t", b=2, t=4)
                    sq_t = sq_p.tile([P, UNITS_PER_DRAIN * 8], f32, tag="sqtile")
                    if g == 0:
                        ph_t = ph_p.tile([P, MEGA_COLS], f32)
                    nc.vector.tensor_mul(s4, mu4, ps4[:, :, :, 4:8])
                    guard[g % 2] = nc.vector.tensor_add(s4, s4, ps4[:, :, :, 0:4])
                    nc.scalar.square(sq_t[:], s_t[:])
                    sq4 = sq_t[:].rearrange("p (c t) -> p c t", t=4)
                    pr_t = sq_p.tile([P, UNITS_PER_DRAIN * 4], f32, tag="prtile")
                    pr3 = pr_t[:].rearrange("p (c t) -> p c t", t=2)
                    nc.vector.tensor_add(pr3, sq4[:, :, 0:2], sq4[:, :, 2:4])
                    nc.vector.tensor_add(ph_t[:, cs], pr3[:, :, 0], pr3[:, :, 1])
                    if g == N_DRAIN - 1:
                        nc.sync.dma_start(ph_v[mt], ph_t[:])

        if rep_cm is not None:
            rep_cm.__exit__(None, None, None)

    nc.compile()
    return nc


def _get_compiled():
    if "nc" not in _CACHE:
        _CACHE["nc"] = _build_nc()
    return _CACHE["nc"]


def _make_in_maps(mu, wavelength, radius, m_real, m_imag):
    C = _fit_coeffs(wavelength, radius, m_real, m_imag)
    cpk = np.zeros((P, 16), np.float16)
    cpk[0:NFEAT, 0:8] = C.astype(np.float16)
    cpk[NFEAT:2 * NFEAT, 8:16] = C.astype(np.float16)
    shards = mu.reshape(N_CORES, PER_CORE)
    return [{"mu": shards[i], "cpk": cpk} for i in range(N_CORES)]


def kernel(cos_theta, wavelength, radius, m_real, m_imag):
    from concourse.bass_utils import run_bass_kernel_spmd

    mu = np.asarray(cos_theta, np.float32).reshape(-1)
    assert mu.size == N_ANGLES
    in_maps = _make_in_maps(mu, float(np.asarray(wavelength)),
                            float(np.asarray(radius)),
                            float(np.asarray(m_real)),
                            float(np.asarray(m_imag)))
    nc = _get_compiled()
    import os
    trace = bool(os.environ.get("MIE_TRACE"))
    res = run_bass_kernel_spmd(nc, in_maps, list(range(N_CORES)), trace=trace)
    _CACHE["last_res"] = res
    out = np.concatenate([np.asarray(res.results[i]["phase"], np.float32)
                          for i in range(N_CORES)])
    return out
